# revision 15
# baseline (speedup 1.0000x reference)
import sys

if "/opt/trn_rl_repo" not in sys.path:
    sys.path.insert(0, "/opt/trn_rl_repo")

import numpy as np

from concourse import bacc, bass, mybir, tile

F32 = mybir.dt.float32
F32R = mybir.dt.float32r
BF16 = mybir.dt.bfloat16

B, T, C = 4, 2048, 1024
H, D = 16, 64
HL = 8          # local heads per core (head-group of 8)
SCALE = float(C) ** -0.5  # 1/32


def _view3(ap2, d1, d2):
    """View a contiguous [P, d1*d2] AP as [P, d1, d2]."""
    return bass.AP(
        tensor=ap2.tensor,
        offset=ap2.offset,
        ap=[ap2.ap[0], [d2, d1], [1, d2]],
    )


def _bcast_rows(ap_rows, reps, free):
    """[2, free] AP -> [2*reps, free] AP replicating each row `reps` times."""
    return bass.AP(
        tensor=ap_rows.tensor,
        offset=ap_rows.offset,
        ap=[[ap_rows.ap[0][0], 2], [0, reps], [1, free]],
    )


def _emit(nc, tc, xT_d, wq_d, wk_d, wv_d, wp_d, y_d):
    r = F32R

    with tc.tile_pool(name="persist", bufs=1) as pp:
        QT = [pp.tile([128, T], F32R, name=f"qt{p}") for p in range(4)]
        KT = [pp.tile([128, T], F32R, name=f"kt{p}") for p in range(4)]
        VP = [pp.tile([128, HL, D + 1], F32R, name=f"vp{s}") for s in range(16)]

        for st in range(16):
            nc.gpsimd.memset(VP[st][:, :, D : D + 1].bitcast(F32), 1.0)

        # ---------------- Phase A: QKV projections ----------------
        with tc.tile_pool(name="xw", bufs=1) as xw, tc.tile_pool(
            name="paps", bufs=1, space="PSUM"
        ) as paps:
            xts = [xw.tile([128, T], BF16, name=f"xt{ct}") for ct in range(8)]
            for ct in range(8):
                nc.sync.dma_start(xts[ct][:], xT_d[ct * 128 : (ct + 1) * 128, :])

            def load_w(wd, tag):
                wt = [xw.tile([128, 512], BF16, name=f"w{tag}{ct}") for ct in range(8)]
                for ct in range(8):
                    nc.sync.dma_start(wt[ct][:], wd[ct * 128 : (ct + 1) * 128, :])
                return wt

            def qk_wave(wt, out_tiles, tbs):
                ps = {}
                for p in range(4):
                    for tb in tbs:
                        ps[(p, tb)] = paps.tile([128, 512], F32, name=f"ps{p}_{tb % 2}")
                for ct in range(8):
                    for p in range(4):
                        for tb in tbs:
                            nc.tensor.matmul(
                                ps[(p, tb)][:],
                                wt[ct][:, p * 128 : (p + 1) * 128],
                                xts[ct][:, tb * 512 : (tb + 1) * 512],
                                start=(ct == 0),
                                stop=(ct == 7),
                            )
                for p in range(4):
                    for tb in tbs:
                        nc.vector.tensor_copy(
                            out_tiles[p][:, tb * 512 : (tb + 1) * 512], ps[(p, tb)][:]
                        )

            def v_wave(wt, sts):
                ps = {}
                for st in sts:
                    ps[st] = paps.tile([128, 512], F32, name=f"ps{(st % 8) // 2}_{st % 2}")
                for ct in range(8):
                    for st in sts:
                        nc.tensor.matmul(
                            ps[st][:],
                            xts[ct][:, st * 128 : (st + 1) * 128],
                            wt[ct][:],
                            start=(ct == 0),
                            stop=(ct == 7),
                        )
                for st in sts:
                    nc.vector.tensor_copy(
                        VP[st][:, :, 0:D], _view3(ps[st][:], HL, D)
                    )

            wqt = load_w(wq_d, "q")
            wkt = load_w(wk_d, "k")
            wvt = load_w(wv_d, "v")
            qk_wave(wqt, QT, (0, 1))
            qk_wave(wqt, QT, (2, 3))
            qk_wave(wkt, KT, (0, 1))
            qk_wave(wkt, KT, (2, 3))
            v_wave(wvt, range(0, 8))
            v_wave(wvt, range(8, 16))

        # ---------------- Attention ----------------
        with tc.tile_pool(name="otp", bufs=1) as otp:
            OT = [otp.tile([128, T], F32R, name=f"ot{p}") for p in range(4)]

            with tc.tile_pool(name="wproj", bufs=1) as wpp:
                wpt = [wpp.tile([128, 1024], F32R, name=f"wp{p}") for p in range(4)]
                for p in range(4):
                    nc.sync.dma_start(wpt[p][:], wp_d[p * 128 : (p + 1) * 128, :])

                with tc.tile_pool(name="attn", bufs=1) as atp, tc.tile_pool(
                    name="expp", bufs=2
                ) as expp, tc.tile_pool(name="stgp", bufs=3) as stgp, tc.tile_pool(
                    name="scps", bufs=1, space="PSUM"
                ) as scps, tc.tile_pool(
                    name="avps", bufs=1, space="PSUM"
                ) as avps:
                    ones = atp.tile([128, 512], F32R, name="ones")
                    nc.vector.memset(ones[:].bitcast(F32), 1.0)
                    masks = [atp.tile([128, 512], F32R, name=f"mask{m}") for m in range(4)]
                    for m in range(4):
                        # keep where f >= 128*m + partition  (causal diag block)
                        nc.gpsimd.affine_select(
                            masks[m][:],
                            ones[:],
                            [[1, 512]],
                            mybir.AluOpType.is_ge,
                            0.0,
                            base=-128 * m,
                            channel_multiplier=-1,
                        )
                    denom = atp.tile([128, T], F32R, name="denom")

                    ALLCH = []
                    for lh in range(HL):
                        for st in range(16):
                            tbs = list(range(st // 4, 4))
                            for c0 in range(0, len(tbs), 2):
                                ALLCH.append((lh, st, tbs[c0 : c0 + 2], c0 == 0))

                    sc_t = {}

                    def issue_scores(i):
                        lh, st, tbs, _ = ALLCH[i]
                        p, half = lh // 2, lh % 2
                        hs = slice(half * 64, (half + 1) * 64)
                        sct = scps.tile([128, 1024], F32, name=f"sc{i % 2}")
                        sc_t[i] = sct
                        for j, tb in enumerate(tbs):
                            nc.tensor.matmul(
                                sct[:, j * 512 : (j + 1) * 512],
                                KT[p][hs, st * 128 : (st + 1) * 128],
                                QT[p][hs, tb * 512 : (tb + 1) * 512],
                                start=True,
                                stop=True,
                            )

                    av = None
                    issue_scores(0)
                    for i, (lh, st, tbs, first) in enumerate(ALLCH):
                        p, half = lh // 2, lh % 2
                        hs = slice(half * 64, (half + 1) * 64)
                        if st == 0 and first:
                            av = [
                                avps.tile([128, 512], F32, name=f"av{tb}")
                                for tb in range(4)
                            ]
                        w = len(tbs) * 512
                        ex = expp.tile([128, 1024], F32R, name=f"ex{i % 2}")
                        nc.scalar.activation(
                            ex[:, 0:w],
                            sc_t.pop(i)[:, 0:w],
                            mybir.ActivationFunctionType.Exp,
                            scale=SCALE,
                        )
                        if i + 1 < len(ALLCH):
                            issue_scores(i + 1)
                        if first:
                            nc.vector.tensor_mul(
                                ex[:, 0:512], ex[:, 0:512], masks[st % 4][:]
                            )
                        for j, tb in enumerate(tbs):
                            nc.tensor.matmul(
                                av[tb][0 : D + 1, :],
                                VP[st][:, lh, :],
                                ex[:, j * 512 : (j + 1) * 512],
                                start=(st == 0),
                                stop=(st == 4 * tb + 3),
                            )
                            if st == 4 * tb + 3:
                                stg = stgp.tile([128, 512], F32R, name="stg")
                                nc.vector.tensor_copy(
                                    stg[0 : D + 1, :], av[tb][0 : D + 1, :]
                                )
                                nc.sync.dma_start(
                                    OT[p][hs, tb * 512 : (tb + 1) * 512],
                                    stg[0:D, :],
                                )
                                nc.sync.dma_start(
                                    denom[lh : lh + 1, tb * 512 : (tb + 1) * 512],
                                    stg[D : D + 1, :],
                                )

                    # normalize: OT[p] *= 1/denom broadcast over 64 partitions/head
                    nc.vector.reciprocal(
                        denom[0:HL, :].bitcast(F32), denom[0:HL, :].bitcast(F32)
                    )
                    with tc.tile_pool(name="rcpp", bufs=1) as rcpp:
                        for p in range(4):
                            rcp = rcpp.tile([128, T], F32R, name="rcp")
                            nc.sync.dma_start(
                                rcp[:], _bcast_rows(denom[2 * p : 2 * p + 2, :], 64, T)
                            )
                            nc.vector.tensor_mul(OT[p][:], OT[p][:], rcp[:])

                # ---------------- Output projection ----------------
                with tc.tile_pool(name="ysb", bufs=4) as ysb, tc.tile_pool(
                    name="pps", bufs=4, space="PSUM"
                ) as pps:
                    for ts in range(16):
                        for cb in range(2):
                            ps = pps.tile([128, 512], F32, name="yp")
                            for p in range(4):
                                nc.tensor.matmul(
                                    ps[:],
                                    OT[p][:, ts * 128 : (ts + 1) * 128],
                                    wpt[p][:, cb * 512 : (cb + 1) * 512],
                                    start=(p == 0),
                                    stop=(p == 3),
                                )
                            yt = ysb.tile([128, 512], F32, name="ys")
                            nc.vector.tensor_copy(yt[:], ps[:])
                            nc.sync.dma_start(
                                y_d[ts * 128 : (ts + 1) * 128, cb * 512 : (cb + 1) * 512],
                                yt[:],
                            )


def _build():
    nc = bacc.Bacc("TRN2", target_bir_lowering=False, debug=False)
    xT_d = nc.dram_tensor("xT", [C, T], BF16, kind="ExternalInput")
    wq_d = nc.dram_tensor("wq", [C, 512], BF16, kind="ExternalInput")
    wk_d = nc.dram_tensor("wk", [C, 512], BF16, kind="ExternalInput")
    wv_d = nc.dram_tensor("wv", [C, 512], BF16, kind="ExternalInput")
    wp_d = nc.dram_tensor("wp", [512, C], F32R, kind="ExternalInput")
    y_d = nc.dram_tensor("y", [T, C], F32, kind="ExternalOutput")
    with tile.TileContext(nc) as tc:
        _emit(nc, tc, xT_d, wq_d, wk_d, wv_d, wp_d, y_d)
    nc.compile()
    return nc


_PROG = None
LAST_EXEC_NS = None
TRACE = False


def _get_prog():
    global _PROG
    if _PROG is None:
        _PROG = _build()
    return _PROG


def kernel(x, Wq, Wk, Wv, Wp, bp):
    global LAST_EXEC_NS
    import ml_dtypes

    from concourse.bass_utils import run_bass_kernel_spmd

    BF = ml_dtypes.bfloat16
    nc = _get_prog()
    x = np.asarray(x, np.float32)
    in_maps = []
    for core in range(8):
        b, g = divmod(core, 2)
        hsl = slice(g * 8, (g + 1) * 8)
        in_maps.append(
            {
                "xT": np.ascontiguousarray(x[b].T).astype(BF),
                "wq": np.ascontiguousarray(
                    np.asarray(Wq, np.float32)[hsl].transpose(1, 0, 2).reshape(C, 512)
                ).astype(BF),
                "wk": np.ascontiguousarray(
                    np.asarray(Wk, np.float32)[hsl].transpose(1, 0, 2).reshape(C, 512)
                ).astype(BF),
                "wv": np.ascontiguousarray(
                    np.asarray(Wv, np.float32)[hsl].transpose(1, 0, 2).reshape(C, 512)
                ).astype(BF),
                "wp": np.ascontiguousarray(np.asarray(Wp, np.float32)[g * 512 : (g + 1) * 512, :]),
            }
        )
    res = run_bass_kernel_spmd(nc, in_maps, list(range(8)), trace=TRACE)
    LAST_EXEC_NS = res.exec_time_ns
    outs = res.results
    y = np.empty((B, T, C), np.float32)
    bpf = np.asarray(bp, np.float32)
    for b in range(B):
        y[b] = outs[2 * b]["y"] + outs[2 * b + 1]["y"] + bpf[None, :]
    return y
